# revision 1
# baseline (speedup 1.0000x reference)
"""Trainium2 Bass kernel for nn_Memory (attention-over-memory with full
softmax-score output).

reference:
    p   = softmax_m(mk^T qk / sqrt(Dk))   # [B, Lm, Lq], softmax over m
    mem = mv @ p                          # [B, Dv, Lq] -> [B, Dv, Hq, Wq]
    returns (mem, p)

Shapes (hardcoded): B=4, Dk=128, Dv=512, Lm=Lq=4096.

Sharding: 8 cores = (batch b = core//2) x (q-half = core%2, Lq_shard=2048).

Per-core kernel, [m, q] layout (m on partitions):
  - scores s[m,q] tile = mk[:,m-tile]^T @ qk_chunk        (fp32r matmul)
  - exp via ScalarE activation (scale = 1/sqrt(Dk) folded in)
  - softmax denominator over m (partition axis) via ones^T @ exp matmul,
    accumulated across the 32 m-tiles in PSUM
  - mem[v,q] += mvT[m-tile, v-tile]^T @ exp  (accumulated over m-tiles)
  - reciprocal on VectorE, broadcast across partitions via K=1 matmul
  - p = exp * recip (VectorE), DMA out

m_v is passed transposed (host-side marshaling) so the stationary operand
of the mem matmul is directly [m, v] in SBUF.
"""
import math
import numpy as np

import concourse.bass as bass
import concourse.mybir as mybir
import concourse.tile as tile
from concourse import bacc
from concourse.bass_utils import run_bass_kernel_spmd

F32 = mybir.dt.float32
F32R = mybir.dt.float32r

B, Dk, Dv, Lm, Lq = 4, 128, 512, 4096, 4096
N_CORES = 8
LQ_SHARD = Lq // 2          # 2048 per core
Q_CHUNK = 512
N_QCHUNKS = LQ_SHARD // Q_CHUNK     # 4
N_MTILES = Lm // 128                # 32
N_VTILES = Dv // 128                # 4

_nc_cache = None
LAST_RESULT = None


def _maybe_register_ntff_hook():
    """Best-effort registration of the axon NTFF profile hook so that
    trace=True (BASS_TRACE=1) yields exec_time_ns. Harmless if unavailable."""
    import sys, types
    if "antenv.axon_hooks" in sys.modules:
        return
    try:
        m = types.ModuleType("antenv.axon_hooks")
        m._hook = None
        m.set_axon_ntff_profile_hook = lambda h: setattr(m, "_hook", h)
        m.get_axon_ntff_profile_hook = lambda: m._hook
        from trn_agent_boot.trn_boot import _ntff_profile_via_ctypes
        hook = _ntff_profile_via_ctypes("/opt/axon/libaxon_pjrt.so")
        sys.modules["antenv.axon_hooks"] = m
        m.set_axon_ntff_profile_hook(hook)
    except Exception:
        pass


def _build():
    nc = bacc.Bacc(target_bir_lowering=False, trn_type="TRN2")

    mk_d = nc.declare_dram_parameter("mk", [Dk, Lm], F32, isOutput=False)
    qk_d = nc.declare_dram_parameter("qk", [Dk, LQ_SHARD], F32, isOutput=False)
    mvt_d = nc.declare_dram_parameter("mvt", [128, N_MTILES * Dv], F32, isOutput=False)
    onc_d = nc.declare_dram_parameter("ones_col", [128, 1], F32, isOutput=False)
    onr_d = nc.declare_dram_parameter("ones_row", [1, 128], F32, isOutput=False)
    p_d = nc.declare_dram_parameter("p", [Lm, LQ_SHARD], F32, isOutput=True)
    mem_d = nc.declare_dram_parameter("mem", [Dv, LQ_SHARD], F32, isOutput=True)

    scale = 1.0 / math.sqrt(Dk)
    EXPF = mybir.ActivationFunctionType.Exp

    with tile.TileContext(nc) as tc:
        with (
            tc.tile_pool(name="inp", bufs=1) as inp,
            tc.tile_pool(name="expp", bufs=1) as expp,
            tc.tile_pool(name="outp", bufs=6) as outp,
            tc.tile_pool(name="small", bufs=2) as small,
            tc.tile_pool(name="ps", bufs=2, space="PSUM") as ps,
            tc.tile_pool(name="ps_acc", bufs=1, space="PSUM") as ps_acc,
        ):
            # ---- input loads (split into ~1-2MB DMAs for queue parallelism)
            qk_sb = inp.tile([128, LQ_SHARD], F32R)
            nc.sync.dma_start(out=qk_sb[:, 0:1024],
                              in_=qk_d[:, 0:1024].bitcast(F32R))
            nc.sync.dma_start(out=qk_sb[:, 1024:2048],
                              in_=qk_d[:, 1024:2048].bitcast(F32R))
            mk_sb = inp.tile([128, Lm], F32R)
            for i in range(4):
                sl = slice(i * 1024, (i + 1) * 1024)
                nc.sync.dma_start(out=mk_sb[:, sl], in_=mk_d[:, sl].bitcast(F32R))
            onc_sb = inp.tile([128, 1], F32R)
            nc.sync.dma_start(out=onc_sb, in_=onc_d[:, :].bitcast(F32R))
            onr_sb = inp.tile([1, 128], F32)
            nc.sync.dma_start(out=onr_sb, in_=onr_d[:, :])
            mvt_sb = inp.tile([128, N_MTILES * Dv], F32R)
            for i in range(8):
                sl = slice(i * 2048, (i + 1) * 2048)
                nc.sync.dma_start(out=mvt_sb[:, sl], in_=mvt_d[:, sl].bitcast(F32R))

            def mvt_slice(mi, vt):
                off = mi * Dv + vt * 128
                return mvt_sb[:, off:off + 128]

            for qc in range(N_QCHUNKS):
                qs = slice(qc * Q_CHUNK, (qc + 1) * Q_CHUNK)

                exp_tiles = [
                    expp.tile([128, Q_CHUNK], F32R, tag=f"exp{mi}", name=f"exp{mi}")
                    for mi in range(N_MTILES)
                ]
                mem_ps = [
                    ps_acc.tile([128, Q_CHUNK], F32, tag=f"mem{vt}", name=f"mem{vt}")
                    for vt in range(N_VTILES)
                ]
                den_ps = ps_acc.tile([1, Q_CHUNK], F32, tag="den")

                def mem_den(mi):
                    for vt in range(N_VTILES):
                        nc.tensor.matmul(
                            mem_ps[vt], mvt_slice(mi, vt), exp_tiles[mi],
                            start=(mi == 0), stop=(mi == N_MTILES - 1),
                        )
                    nc.tensor.matmul(
                        den_ps, onc_sb, exp_tiles[mi],
                        start=(mi == 0), stop=(mi == N_MTILES - 1),
                    )

                # software-pipelined: scores(mi) then mem/den(mi-1)
                for mi in range(N_MTILES):
                    s_ps = ps.tile([128, Q_CHUNK], F32, tag="scores")
                    nc.tensor.matmul(
                        s_ps, mk_sb[:, mi * 128:(mi + 1) * 128], qk_sb[:, qs],
                        start=True, stop=True,
                    )
                    nc.scalar.activation(exp_tiles[mi], s_ps, EXPF,
                                         bias=0.0, scale=scale)
                    if mi >= 1:
                        mem_den(mi - 1)
                mem_den(N_MTILES - 1)

                # denominator -> reciprocal -> partition broadcast
                recip_sb = small.tile([1, Q_CHUNK], F32, tag="recip")
                nc.vector.reciprocal(recip_sb, den_ps)
                bcast_ps = ps.tile([128, Q_CHUNK], F32, tag="scores", name="bcast_ps")
                nc.tensor.matmul(bcast_ps, onr_sb, recip_sb, start=True, stop=True)
                bcast_sb = small.tile([128, Q_CHUNK], F32, tag="bcast_sb")
                nc.scalar.copy(bcast_sb, bcast_ps)

                # normalize p and write out
                for mi in range(N_MTILES):
                    pn = outp.tile([128, Q_CHUNK], F32, tag="pn")
                    nc.vector.tensor_mul(pn, exp_tiles[mi].bitcast(F32), bcast_sb)
                    nc.sync.dma_start(
                        out=p_d[mi * 128:(mi + 1) * 128, qs], in_=pn)

                # normalize mem and write out
                for vt in range(N_VTILES):
                    mn = outp.tile([128, Q_CHUNK], F32, tag="mn")
                    nc.vector.tensor_mul(mn, mem_ps[vt], bcast_sb)
                    nc.sync.dma_start(
                        out=mem_d[vt * 128:(vt + 1) * 128, qs], in_=mn)

    nc.compile()
    return nc


def _get_nc():
    global _nc_cache
    if _nc_cache is None:
        _nc_cache = _build()
    return _nc_cache


def kernel(m_k, m_v, q_k):
    global LAST_RESULT
    _maybe_register_ntff_hook()

    m_k = np.ascontiguousarray(np.asarray(m_k, dtype=np.float32)).reshape(B, Dk, Lm)
    m_v = np.ascontiguousarray(np.asarray(m_v, dtype=np.float32)).reshape(B, Dv, Lm)
    q_k = np.ascontiguousarray(np.asarray(q_k, dtype=np.float32)).reshape(B, Dk, Lq)

    ones_col = np.ones((128, 1), np.float32)
    ones_row = np.ones((1, 128), np.float32)

    # mvT packed per batch: [Lm, Dv] -> [N_MTILES, 128, Dv] -> [128, N_MTILES*Dv]
    mvt_packed = []
    for b in range(B):
        mvT = m_v[b].T  # [Lm, Dv]
        mvt_packed.append(np.ascontiguousarray(
            mvT.reshape(N_MTILES, 128, Dv).transpose(1, 0, 2)
            .reshape(128, N_MTILES * Dv)))

    in_maps = []
    for core in range(N_CORES):
        b, half = core // 2, core % 2
        qsl = slice(half * LQ_SHARD, (half + 1) * LQ_SHARD)
        in_maps.append({
            "mk": m_k[b],
            "qk": np.ascontiguousarray(q_k[b][:, qsl]),
            "mvt": mvt_packed[b],
            "ones_col": ones_col,
            "ones_row": ones_row,
        })

    nc = _get_nc()
    res = run_bass_kernel_spmd(nc, in_maps, core_ids=list(range(N_CORES)))
    LAST_RESULT = res

    p_full = np.empty((B, Lm, Lq), np.float32)
    mem_full = np.empty((B, Dv, Lq), np.float32)
    for core in range(N_CORES):
        b, half = core // 2, core % 2
        qsl = slice(half * LQ_SHARD, (half + 1) * LQ_SHARD)
        p_full[b][:, qsl] = res.results[core]["p"]
        mem_full[b][:, qsl] = res.results[core]["mem"]

    return mem_full.reshape(B, Dv, 64, 64), p_full
